# revision 17
# baseline (speedup 1.0000x reference)
"""NT-Xent loss kernel for Trainium2 (8 NeuronCores, data-parallel over N).

Inputs: zis, zjs [N=128, B=256, D=128] fp32.
Per sample: reps = concat(zjs[n], zis[n]) -> [512, 128]; cosine similarity
matrix S = normalize(reps) @ normalize(reps).T; loss contribution per row k:
logsumexp_{j!=k}(S[k,j]/T) - S[k,(k+B)%2B]/T, with T=0.5.

Device strategy (per core, 16 samples), v2 — upper-block-triangle:
  - S is symmetric, so only the 10 upper-triangle 128x128 blocks (of 4x4)
    are computed: sim chunk m covers columns j >= 128m (N = (4-m)*128)
  - exp(2*sim) on ACT with accum_out: the fused per-partition accumulator
    gives each chunk's row sums over the upper region for free (rs1)
  - the missing lower-region contributions are column sums of the 6
    off-diagonal exp blocks: 3 one-hot matmuls per sample accumulate them
    into a shared [16, 384] PSUM tile (rs2), recombined on host
  - row sum-of-squares via fused tensor_tensor_reduce (one DVE pass/chunk)
  - rsqrt = Exp(-0.5*Ln(ssq)) on ACT, batched over sample groups
  - pos terms via tensor_tensor_reduce accum -> pos_acc[:, n] (no PE work)
Host: rs[n,m,p] = rs1[p,4n+m] + rs2[n,(m-1)*128+p]; lse = log(rs - e^2).
"""

import os
import sys

import numpy as np
import ml_dtypes

if "/opt/trn_rl_repo" not in sys.path:
    sys.path.insert(0, "/opt/trn_rl_repo")

N_CORES = 8
N_FULL, B, D = 128, 256, 128
SPC = N_FULL // N_CORES  # samples per core = 16
TWO_B = 2 * B  # 512
N_CHUNKS = 4  # 512 rows / 128 partitions
TEMP = 0.5
GROUPS = [[0], [1], [2, 3], [4, 5, 6, 7], [8, 9, 10, 11], [12, 13, 14, 15]]
# e-scratch layout: chunk m's upper region, width (4-m)*128
E_OFF = [0, 512, 896, 1152]
E_W = [512, 384, 256, 128]

_compiled = None


def _build():
    import concourse.bacc as bacc
    import concourse.tile as tile
    import concourse.mybir as mybir

    f32 = mybir.dt.float32
    bf16 = mybir.dt.bfloat16
    AF = mybir.ActivationFunctionType
    OP = mybir.AluOpType

    loop_n = int(os.environ.get("KLOOP", "1"))

    nc = bacc.Bacc(
        "TRN2",
        target_bir_lowering=False,
        debug=False,
        enable_asserts=False,
        num_devices=N_CORES,
    )

    zjs_d = nc.dram_tensor("zjs", [SPC, B, D], f32, kind="ExternalInput")
    zis_d = nc.dram_tensor("zis", [SPC, B, D], f32, kind="ExternalInput")
    oh_d = nc.dram_tensor("ohstrip", [128, 127], bf16, kind="ExternalInput")
    ident_d = nc.dram_tensor("ident", [128, 128], bf16, kind="ExternalInput")
    rs1_d = nc.dram_tensor("rs1_out", [128, SPC * N_CHUNKS], f32, kind="ExternalOutput")
    rs2_d = nc.dram_tensor("rs2_out", [SPC, 3 * 128], f32, kind="ExternalOutput")
    pos_d = nc.dram_tensor("pos_out", [128, SPC], f32, kind="ExternalOutput")

    with tile.TileContext(nc) as tc:
        # One ACT table set covers both Ln and Exp; preloading it here keeps
        # bacc's table-load pass from ping-ponging between the ln-only and
        # exp-only sets.
        from concourse.hw_specs import get_activation_tables

        tabs = list(get_activation_tables(nc.m.arch).keys())
        nc.scalar.add_instruction(
            mybir.InstLoadActFuncSet(
                name=nc.get_next_instruction_name(),
                ins=[],
                outs=[],
                act_func_set_id=tabs.index("natural_log_exp_and_others"),
            )
        )

        with (
            tc.tile_pool(name="raw", bufs=4) as rawp,
            tc.tile_pool(name="scratch", bufs=2) as scrp,
            tc.tile_pool(name="grp", bufs=2) as grpp,
            tc.tile_pool(name="rhat", bufs=3) as rhatp,
            tc.tile_pool(name="that", bufs=2) as thatp,
            tc.tile_pool(name="ework", bufs=2) as ep,
            tc.tile_pool(name="singles", bufs=1) as singles,
            tc.tile_pool(name="psA", bufs=2, space="PSUM") as psA_pool,
            tc.tile_pool(name="psB", bufs=2, space="PSUM") as psB_pool,
            tc.tile_pool(name="psC", bufs=2, space="PSUM") as psC_pool,
            tc.tile_pool(name="pt", bufs=1, space="PSUM") as pt_pool,
            tc.tile_pool(name="prs2", bufs=1, space="PSUM") as prs2_pool,
        ):
            oh_sb = singles.tile([128, 127], bf16)
            nc.sync.dma_start(out=oh_sb, in_=oh_d.ap())
            ident_sb = singles.tile([128, 128], bf16)
            nc.sync.dma_start(out=ident_sb, in_=ident_d.ap())

            def body():
                rs1_sb = singles.tile([128, SPC * N_CHUNKS], f32, name="rs1_sb")
                pos_sb = singles.tile([128, SPC], f32, name="pos_sb")
                # full bank so no other PSUM pool shares it
                rs2_ps = prs2_pool.tile([128, 512], f32, name="rs2_ps")
                raw_tiles = {}
                stat_tiles = {}

                def load_quad(q):
                    """One 512KB DMA per source tensor for samples 4q..4q+3;
                    zjs rides the SP HWDGE ring, zis the ACT ring, so both
                    rings stream in parallel. Layout [p, src, n, c, d] keeps
                    each source's destination region contiguous."""
                    t = rawp.tile(
                        [128, 2, 4, 2, D], f32, tag="quad", name=f"q_{q}"
                    )
                    for h, (src, eng) in enumerate(((zjs_d, nc.sync), (zis_d, nc.scalar))):
                        eng.dma_start(
                            out=t[:, h, :, :, :],
                            in_=src.ap()[4 * q : 4 * q + 4].rearrange(
                                "n (c p) d -> p n c d", p=128
                            ),
                        )
                    raw_tiles[q] = t

                def chunk_ap(n, c):
                    return raw_tiles[n // 4][:, c // 2, n % 4, c % 2, :]

                def group_ssq_tile(gi):
                    t = grpp.tile(
                        [128, len(GROUPS[gi]) * N_CHUNKS],
                        f32,
                        tag="gssq",
                        name=f"gq_{gi}",
                    )
                    stat_tiles[gi] = t

                def ssq_sample(n, gi, k):
                    """Row sum-of-squares per row chunk -> group tile cols.
                    (tensor_tensor_reduce crashes this HW path; use the
                    plain mul + reduce pair.)"""
                    sample = raw_tiles[n // 4][:, :, n % 4, :, :]
                    sq_scr = scrp.tile(
                        [128, N_CHUNKS, D], f32, tag="sq", name=f"sq_{n}"
                    )
                    nc.vector.tensor_mul(
                        sq_scr.rearrange("p (h c) d -> p h c d", h=2), sample, sample
                    )
                    nc.vector.tensor_reduce(
                        out=stat_tiles[gi][:, k * N_CHUNKS : (k + 1) * N_CHUNKS],
                        in_=sq_scr,
                        axis=mybir.AxisListType.X,
                        op=OP.add,
                    )

                def rsqrt_group(gi):
                    """scale[:, k*4+c] = ssq^-0.5 for group gi."""
                    grp = GROUPS[gi]
                    ssq_t = stat_tiles.pop(gi)
                    nc.vector.tensor_scalar_max(ssq_t, ssq_t, 1e-16)
                    ln_t = grpp.tile(
                        [128, len(grp) * N_CHUNKS], f32, tag="ln", name=f"ln_{gi}"
                    )
                    nc.scalar.activation(out=ln_t, in_=ssq_t, func=AF.Ln)
                    sc_t = grpp.tile(
                        [128, len(grp) * N_CHUNKS], f32, tag="sc", name=f"sc_{gi}"
                    )
                    nc.scalar.activation(out=sc_t, in_=ln_t, func=AF.Exp, scale=-0.5)
                    return sc_t

                deferred = []  # colsum matmuls of the previous sample

                def flush_deferred():
                    while deferred:
                        deferred.pop(0)()

                def main_sample(n, k, sc_t):
                    rhat = rhatp.tile(
                        [128, N_CHUNKS, D], bf16, tag="rhat", name=f"rh_{n}"
                    )
                    for c in range(N_CHUNKS):
                        idx = k * N_CHUNKS + c
                        nc.vector.tensor_scalar_mul(
                            rhat[:, c, :], chunk_ap(n, c), sc_t[:, idx : idx + 1]
                        )

                    # full-bank PSUM tile (1024 bf16 = 2KB) so no other
                    # pool shares this bank (PE-W + DVE-R same bank is a
                    # fatal PSUM collision); transposes use the first half.
                    tpad = pt_pool.tile([128, 2 * N_CHUNKS, 128], bf16, tag="tps")
                    tpsum = tpad[:, 0:N_CHUNKS, :]
                    for c in range(N_CHUNKS):
                        nc.tensor.transpose(
                            out=tpsum[:, c, :], in_=rhat[:, c, :], identity=ident_sb
                        )
                    that = thatp.tile([128, N_CHUNKS * 128], bf16, tag="that")
                    nc.vector.tensor_copy(
                        out=that, in_=tpsum.rearrange("p c d -> p (c d)")
                    )

                    # pos: product of transposed halves, reduced to a column
                    pos_scr = scrp.tile([128, B], bf16, tag="pos", name=f"ps_{n}")
                    nc.vector.tensor_mul(pos_scr, that[:, 0:B], that[:, B : 2 * B])
                    nc.vector.tensor_reduce(
                        out=pos_sb[:, n : n + 1],
                        in_=pos_scr,
                        axis=mybir.AxisListType.X,
                        op=OP.add,
                    )

                    # upper-triangle sim: chunk m covers cols j >= 128m.
                    # Bank packing (full 2KB banks): A = c0 (512); B = c1
                    # (384) + c3 (128); C = c2 (256, rest unused). Within
                    # bank B: matmul c3 issues BEFORE c1, and exp c1 before
                    # exp c3 (ACT FIFO), so no exp of bank B can overlap a
                    # PE write to it (fatal PSUM collision otherwise).
                    psA = psA_pool.tile([128, 512], f32, tag="psA", name=f"pA_{n}")
                    psB = psB_pool.tile([128, 512], f32, tag="psB", name=f"pB_{n}")
                    psC = psC_pool.tile([128, 512], f32, tag="psC", name=f"pC_{n}")
                    sim_out = [
                        psA,
                        psB[:, 0:384],
                        psC[:, 0:256],
                        psB[:, 384:512],
                    ]
                    for m in (0, 3, 1, 2):
                        nc.tensor.matmul(
                            out=sim_out[m],
                            lhsT=that[:, m * 128 : (m + 1) * 128],
                            rhs=that[:, m * 128 :],
                            start=True,
                            stop=True,
                        )
                    flush_deferred()

                    # exp(2*sim) per chunk; accum_out = row sums over the
                    # upper region (rs1). e values go to scratch for colsums.
                    e_sb = ep.tile([128, 1280], bf16, tag="e", name=f"e_{n}")
                    for m in (0, 1, 3, 2):
                        kw = (
                            {}
                            if os.environ.get("KNOACC")
                            else {
                                "accum_out": rs1_sb[
                                    :, N_CHUNKS * n + m : N_CHUNKS * n + m + 1
                                ]
                            }
                        )
                        nc.scalar.activation(
                            out=e_sb[:, E_OFF[m] : E_OFF[m] + E_W[m]],
                            in_=sim_out[m],
                            func=AF.Exp,
                            scale=1.0 / TEMP,
                            **kw,
                        )
                    if os.environ.get("KNOACC"):
                        for m in (0, 1, 3, 2):
                            nc.vector.tensor_reduce(
                                out=rs1_sb[
                                    :, N_CHUNKS * n + m : N_CHUNKS * n + m + 1
                                ],
                                in_=e_sb[:, E_OFF[m] : E_OFF[m] + E_W[m]],
                                axis=mybir.AxisListType.X,
                                op=OP.add,
                            )

                    def emit_colsums():
                        # column sums of the 6 off-diagonal blocks: for
                        # chunk c, its off-diag region covers chunks t>c;
                        # out slice offset c*128 aligns block t at column
                        # (t-1)*128 so contributions accumulate per t.
                        for c in range(3):
                            nc.tensor.matmul(
                                out=rs2_ps[0:SPC, c * 128 : 384],
                                lhsT=oh_sb[:, 63 - n : 63 - n + SPC],
                                rhs=e_sb[:, E_OFF[c] + 128 : E_OFF[c] + E_W[c]],
                                start=(n == 0 and c == 0),
                                stop=(n == SPC - 1 and c == 2),
                                skip_group_check=True,
                            )

                    deferred.append(emit_colsums)

                # prologue: all loads upfront (DMA rings run ahead), group 0 prep
                for q in range(SPC // 4):
                    load_quad(q)
                group_ssq_tile(0)
                for k, n in enumerate(GROUPS[0]):
                    ssq_sample(n, 0, k)
                sc_t = rsqrt_group(0)

                for gi, grp in enumerate(GROUPS):
                    nxt = GROUPS[gi + 1] if gi + 1 < len(GROUPS) else None
                    L = len(grp)
                    # spread next group's ssq over this group's early samples
                    prep_slots = [[] for _ in range(L)]
                    if nxt:
                        group_ssq_tile(gi + 1)
                        for j, nn in enumerate(nxt):
                            prep_slots[j % max(L - 1, 1)].append((nn, j))
                    next_sc = None
                    for k, n in enumerate(grp):
                        for nn, j in prep_slots[k]:
                            ssq_sample(nn, gi + 1, j)
                        if nxt and k == max(L - 2, 0):
                            next_sc = rsqrt_group(gi + 1)
                        main_sample(n, k, sc_t)
                    sc_t = next_sc

                flush_deferred()
                rs2_sb = singles.tile([SPC, 3 * 128], f32, name="rs2_sb")
                nc.vector.tensor_copy(out=rs2_sb, in_=rs2_ps[0:SPC, 0 : 3 * 128])
                nc.sync.dma_start(out=rs1_d.ap(), in_=rs1_sb)
                nc.sync.dma_start(out=rs2_d.ap(), in_=rs2_sb)
                nc.sync.dma_start(out=pos_d.ap(), in_=pos_sb)

            if loop_n > 1:
                with tc.For_i(0, loop_n, 1):
                    body()
            else:
                body()

    nc.compile()
    return nc


def _host_constants():
    oh = np.zeros((128, 127), dtype=ml_dtypes.bfloat16)
    oh[:, 63] = 1
    ident = np.eye(128, dtype=ml_dtypes.bfloat16)
    return oh, ident


def kernel(zis, zjs):
    global _compiled
    if _compiled is None:
        _compiled = _build()
    nc = _compiled

    from concourse import bass_utils

    zis = np.ascontiguousarray(np.asarray(zis, dtype=np.float32))
    zjs = np.ascontiguousarray(np.asarray(zjs, dtype=np.float32))
    oh, ident = _host_constants()

    in_maps = []
    for c in range(N_CORES):
        sl = slice(c * SPC, (c + 1) * SPC)
        in_maps.append(
            {
                "zjs": np.ascontiguousarray(zjs[sl]),
                "zis": np.ascontiguousarray(zis[sl]),
                "ohstrip": oh,
                "ident": ident,
            }
        )

    res = bass_utils.run_bass_kernel_spmd(nc, in_maps, core_ids=list(range(N_CORES)))

    total_lse = 0.0
    total_pos = 0.0
    diag = np.exp(np.float64(1.0 / TEMP))
    for r in res.results:
        # rs1[p, 4n+m]: row sums over upper region for row 128m+p of sample n
        rs1 = (
            r["rs1_out"].astype(np.float64).reshape(128, SPC, N_CHUNKS)
        )  # [p, n, m]
        rs = np.transpose(rs1, (1, 2, 0)).copy()  # [n, m, p]
        # rs2[n, (m-1)*128+p]: lower-region colsum contributions for m>=1
        rs2 = r["rs2_out"].astype(np.float64).reshape(SPC, 3, 128)  # [n, m-1, p]
        rs[:, 1:, :] += rs2
        total_lse += np.log(rs - diag).sum()
        total_pos += r["pos_out"].astype(np.float64).sum()

    # sum_k pos_k over all 512 rows = 2 * sum_pairs (cos/TEMP) = 4 * sum(pos)
    loss = (total_lse - (2.0 / TEMP) * total_pos) / TWO_B
    return np.float32(loss)
